# revision 1
# baseline (speedup 1.0000x reference)
"""Trainium2 Bass kernel for nn_DAGController.

Sequential GRU controller: 1128 steps over batch 2048, hidden 64.
Data-parallel over batch across 8 NeuronCores (256 samples/core).

Math recast per step t (per core, h = h_t, dec = dec_{t-1}):
  pre_r  = W_hh[0:64] @ h   + w_ih[0:64,2]  * dec + C_r[t]    (sigmoid)
  pre_z  = W_hh[64:128] @ h + w_ih[64:128,2]* dec + C_z[t]    (sigmoid)
  hn     = W_hh[128:] @ h  (+ b_hh[128:] inside the a-op)
  inn_v  = w_ih[128:,2] * dec                                 (rank-1 matmul)
  n      = tanh( (hn + b_hh_n)*r + inn_v + C_n[t] )
  h_{t+1} = n + z*(h - n)
  logit_t = w_head @ h_{t+1}          (separate M=1 matmul, next iteration)
  dec_t   = logit_t > L[t]   where L = log(u/(1-u)) - b_head  (host precomputed;
            equivalent to u < sigmoid(logit+b_head) by monotonicity)

C_* fold the (compile-time) src/dst embedding scalars and biases.
All compute-op operands live on partitions [0:64] or [0:1] (walrus requires
matching partition ranges across a compute instruction's APs).
Device streams raw logits + decisions to DRAM; host computes log-probs.
"""

import numpy as np

NUM_SPANS = 48
HID = 64
BATCH = 2048
T = 1128  # number of decisions
NCORES = 8
BCORE = BATCH // NCORES          # 256 samples per core
G = 2                            # pointwise groups per core (pipelining)
F = BCORE // G                   # 128 samples per group
EPS = 1e-8

# perf/impl flags
USE_F32R = True      # float32r matmuls (1 cyc/row at N>=256) vs fp32 (4 cyc/row)

_PAIRS = np.array(
    [(i, j) for i in range(NUM_SPANS) for j in range(i + 1, NUM_SPANS)],
    dtype=np.int32,
)

_CACHE = {}


def _build_program(t_steps=T):
    import concourse.bacc as bacc
    import concourse.tile as tile
    import concourse.mybir as mybir

    fp32 = mybir.dt.float32
    mm_dt = mybir.dt.float32r if USE_F32R else mybir.dt.float32
    Alu = mybir.AluOpType
    Act = mybir.ActivationFunctionType

    nc = bacc.Bacc("TRN2", target_bir_lowering=False)

    # ---- DRAM I/O ----
    # lhsT weights, packed: [64, 193] = whn|wr|wz|whead
    d_w = nc.dram_tensor("wpack", [64, 193], mm_dt, kind="ExternalInput").ap()
    # K=1 lhsT rows packed on one partition: [1, 192] = w2n|wrd|wzd
    d_wkd = nc.dram_tensor("wkd", [1, 192], mm_dt, kind="ExternalInput").ap()
    # per-step biases: rows 0:64 C_r, 64:128 C_z, 128:192 C_n  -> 3 tiles
    d_cr = nc.dram_tensor("c_r", [64, t_steps], fp32, kind="ExternalInput").ap()
    d_cz = nc.dram_tensor("c_z", [64, t_steps], fp32, kind="ExternalInput").ap()
    d_cn = nc.dram_tensor("c_n", [64, t_steps], fp32, kind="ExternalInput").ap()
    d_bhn = nc.dram_tensor("bhn", [64, 1], fp32, kind="ExternalInput").ap()
    # decision thresholds [t, j] = L[sample j, step t]
    d_thr = nc.dram_tensor("thr", [t_steps, BCORE], mm_dt, kind="ExternalInput").ap()
    # outputs
    d_dec = nc.dram_tensor("dec_out", [t_steps, BCORE], mm_dt, kind="ExternalOutput").ap()
    d_lgt = nc.dram_tensor("lgt_out", [t_steps, BCORE], fp32, kind="ExternalOutput").ap()

    with tile.TileContext(nc) as tc:
        with (
            tc.tile_pool(name="singles", bufs=1) as singles,
            tc.tile_pool(name="hpool", bufs=4) as hpool,
            tc.tile_pool(name="work", bufs=3) as work,
            tc.tile_pool(name="small", bufs=3) as small,
            tc.tile_pool(name="thrp", bufs=8) as thrp,
            tc.tile_pool(name="decp", bufs=4) as decp,
            tc.tile_pool(name="ps_a", bufs=3, space="PSUM") as ps_a,
            tc.tile_pool(name="ps_b", bufs=3, space="PSUM") as ps_b,
            tc.tile_pool(name="ps_l", bufs=2, space="PSUM") as ps_l,
        ):
            # persistent tiles
            wpack = singles.tile([64, 193], mm_dt)
            whn = wpack[:, 0:64]
            wr = wpack[:, 64:128]
            wz = wpack[:, 128:192]
            whead = wpack[:, 192:193]
            # K=1 lhsT rows live on partition 0: pack [1, 192]: w2n|wrd|wzd
            wkd = singles.tile([1, 192], mm_dt)
            w2n = wkd[0:1, 0:64]
            wrd = wkd[0:1, 64:128]
            wzd = wkd[0:1, 128:192]
            cr_sb = singles.tile([64, t_steps], fp32)
            cz_sb = singles.tile([64, t_steps], fp32)
            cn_sb = singles.tile([64, t_steps], fp32)
            bhn_sb = singles.tile([64, 1], fp32)

            nc.sync.dma_start(out=wpack, in_=d_w)
            nc.sync.dma_start(out=wkd, in_=d_wkd)
            nc.sync.dma_start(out=cr_sb, in_=d_cr)
            nc.sync.dma_start(out=cz_sb, in_=d_cz)
            nc.sync.dma_start(out=cn_sb, in_=d_cn)
            nc.sync.dma_start(out=bhn_sb, in_=d_bhn)

            # initial state: h_0 = 0, dec_{-1} = 0
            h_prev = hpool.tile([64, BCORE], mm_dt, tag="h")
            nc.vector.memset(h_prev.bitcast(fp32), 0.0)
            dec0 = singles.tile([1, BCORE], mm_dt)
            nc.vector.memset(dec0.bitcast(fp32), 0.0)

            dec_prev = dec0
            thr_tiles = {}

            for t in range(t_steps):
                # prefetch threshold row for THIS step (consumed at t+1)
                thr_t = thrp.tile([1, BCORE], mm_dt, tag="thr")
                nc.gpsimd.dma_start(out=thr_t, in_=d_thr[t : t + 1, :])
                thr_tiles[t] = thr_t

                # logit_{t-1} = w_head @ h_t  (M=1 matmul -> psum partition 0)
                psL = ps_l.tile([1, BCORE], fp32, tag="psL")
                nc.tensor.matmul(psL, whead, h_prev, start=True, stop=True)

                if t >= 1:
                    dec_new = decp.tile([1, BCORE], mm_dt, tag="dec")
                    nc.vector.tensor_tensor(
                        out=dec_new, in0=psL, in1=thr_tiles.pop(t - 1), op=Alu.is_gt
                    )
                    nc.gpsimd.dma_start(out=d_dec[t - 1 : t, :], in_=dec_new)
                    lgt_st = decp.tile([1, BCORE], fp32, tag="lgt")
                    nc.scalar.copy(out=lgt_st, in_=psL)
                    nc.gpsimd.dma_start(out=d_lgt[t - 1 : t, :], in_=lgt_st)
                    dec_src = dec_new[0:1, :]
                    dec_prev = dec_new
                else:
                    dec_src = dec_prev[0:1, :]

                # ---- psum bank A: [64, 512]: hn cols 0:256, inn_v cols 256:512
                psA = ps_a.tile([64, 2 * BCORE], fp32, tag="psA")
                nc.tensor.matmul(psA[:, 0:BCORE], whn, h_prev, start=True, stop=True)
                nc.tensor.matmul(
                    psA[:, BCORE : 2 * BCORE], w2n, dec_src, start=True, stop=True
                )

                # ---- psum bank B: [64, 512]: r-pre cols 0:256, z-pre 256:512
                psB = ps_b.tile([64, 2 * BCORE], fp32, tag="psB")
                nc.tensor.matmul(psB[:, 0:BCORE], wr, h_prev, start=True, stop=False)
                nc.tensor.matmul(
                    psB[:, 0:BCORE], wrd, dec_src, start=False, stop=True
                )
                nc.tensor.matmul(
                    psB[:, BCORE : 2 * BCORE], wz, h_prev, start=True, stop=False
                )
                nc.tensor.matmul(
                    psB[:, BCORE : 2 * BCORE], wzd, dec_src, start=False, stop=True
                )

                # ---- pointwise ----
                # sigmoids AB-fused (start of chain, no stagger possible)
                r_sb = work.tile([64, BCORE], fp32, tag="r")
                z_sb = work.tile([64, BCORE], fp32, tag="z")
                n_sb = work.tile([64, BCORE], fp32, tag="n")
                h_new = hpool.tile([64, BCORE], mm_dt, tag="h")
                nc.scalar.activation(
                    out=r_sb, in_=psB[:, 0:BCORE], func=Act.Sigmoid,
                    bias=cr_sb[:, t : t + 1], scale=1.0,
                )
                nc.scalar.activation(
                    out=z_sb, in_=psB[:, BCORE : 2 * BCORE], func=Act.Sigmoid,
                    bias=cz_sb[:, t : t + 1], scale=1.0,
                )
                for g in range(G):
                    gs = slice(g * F, (g + 1) * F)
                    # a = (hn + bhn) * r
                    a_g = small.tile([64, F], fp32, tag="a")
                    nc.vector.scalar_tensor_tensor(
                        out=a_g, in0=psA[:, gs], scalar=bhn_sb[:, 0:1],
                        in1=r_sb[:, gs], op0=Alu.add, op1=Alu.mult,
                    )
                    # npre = inn_v + a
                    npre_g = small.tile([64, F], fp32, tag="npre")
                    nc.vector.tensor_tensor(
                        out=npre_g,
                        in0=psA[:, BCORE + g * F : BCORE + (g + 1) * F],
                        in1=a_g, op=Alu.add,
                    )
                    # n = tanh(npre + C_n[t])
                    nc.scalar.activation(
                        out=n_sb[:, gs], in_=npre_g, func=Act.Tanh,
                        bias=cn_sb[:, t : t + 1], scale=1.0,
                    )
                    # h' = n + z*(h - n)
                    w_g = small.tile([64, F], fp32, tag="w")
                    nc.vector.tensor_tensor(
                        out=w_g, in0=h_prev[:, gs], in1=n_sb[:, gs],
                        op=Alu.subtract,
                    )
                    p_g = small.tile([64, F], fp32, tag="p")
                    nc.vector.tensor_tensor(
                        out=p_g, in0=z_sb[:, gs], in1=w_g, op=Alu.mult,
                    )
                    nc.vector.tensor_tensor(
                        out=h_new[:, gs], in0=n_sb[:, gs], in1=p_g, op=Alu.add,
                    )
                h_prev = h_new

            # final step's logit/dec
            psL = ps_l.tile([1, BCORE], fp32, tag="psL")
            nc.tensor.matmul(psL, whead, h_prev, start=True, stop=True)
            tp = t_steps - 1
            dec_new = decp.tile([1, BCORE], mm_dt, tag="dec")
            nc.vector.tensor_tensor(
                out=dec_new, in0=psL, in1=thr_tiles.pop(tp), op=Alu.is_gt
            )
            nc.gpsimd.dma_start(out=d_dec[tp : tp + 1, :], in_=dec_new)
            lgt_st = decp.tile([1, BCORE], fp32, tag="lgt")
            nc.scalar.copy(out=lgt_st, in_=psL)
            nc.gpsimd.dma_start(out=d_lgt[tp : tp + 1, :], in_=lgt_st)

    nc.compile()
    return nc


def _get_program(t_steps=T):
    key = (t_steps, USE_F32R, G)
    if key not in _CACHE:
        _CACHE[key] = _build_program(t_steps)
    return _CACHE[key]


def _host_prep(embed, w_ih, w_hh, b_ih, b_hh, w_head, b_head, u, t_steps=T):
    """Build per-core input maps."""
    f32 = np.float32
    e0 = np.asarray(embed, f32)[:, 0]
    src_v = e0[_PAIRS[:t_steps, 0]].astype(np.float64)  # (T,)
    dst_v = e0[_PAIRS[:t_steps, 1]].astype(np.float64)
    w_ih = np.asarray(w_ih, f32)
    w_hh = np.asarray(w_hh, f32)
    b_ih = np.asarray(b_ih, f32)
    b_hh = np.asarray(b_hh, f32)
    w_head = np.asarray(w_head, f32)
    bh = float(np.asarray(b_head, f32)[0])
    u = np.asarray(u, f32)[:, :t_steps]

    # per-step biases  (f64 accumulate, f32 store)
    def bias_block(lo, hi, add_bhh):
        b = b_ih[lo:hi].astype(np.float64)
        if add_bhh:
            b = b + b_hh[lo:hi].astype(np.float64)
        return (
            np.outer(w_ih[lo:hi, 0].astype(np.float64), src_v)
            + np.outer(w_ih[lo:hi, 1].astype(np.float64), dst_v)
            + b[:, None]
        ).astype(f32)

    c_r = bias_block(0, 64, True)      # [64, T]
    c_z = bias_block(64, 128, True)    # [64, T]
    c_n = bias_block(128, 192, False)  # [64, T]

    # packed lhsT weights [64, 193]: whn|wr|wz|whead
    wpack = np.zeros((64, 193), f32)
    wpack[:, 0:64] = w_hh[128:192, :].T
    wpack[:, 64:128] = w_hh[0:64, :].T
    wpack[:, 128:192] = w_hh[64:128, :].T
    wpack[:, 192] = w_head
    # [1, 192] = w2n|wrd|wzd
    wkd = np.concatenate(
        [w_ih[128:192, 2], w_ih[0:64, 2], w_ih[64:128, 2]]
    )[None, :].astype(f32)
    bhn = np.ascontiguousarray(b_hh[128:192][:, None]).astype(f32)  # [64,1]

    # thresholds in logit space: L = log(u/(1-u)) - b_head
    u64 = u.astype(np.float64)
    with np.errstate(divide="ignore"):
        L = np.log(u64) - np.log1p(-u64) - bh  # [B, T]; -inf at u=0
    L = L.astype(f32)

    in_maps = []
    for c in range(NCORES):
        Lc = L[c * BCORE : (c + 1) * BCORE, :]  # [256, T]
        thr = np.ascontiguousarray(Lc.T)  # [T, 256]
        in_maps.append(
            {
                "wpack": wpack,
                "wkd": wkd,
                "c_r": np.ascontiguousarray(c_r),
                "c_z": np.ascontiguousarray(c_z),
                "c_n": np.ascontiguousarray(c_n),
                "bhn": bhn,
                "thr": thr,
            }
        )
    return in_maps, bh


def _sigmoid_f32(x):
    # numerically stable, f32 in/out (matches jax.nn.sigmoid closely)
    x = x.astype(np.float32)
    out = np.empty_like(x)
    pos = x >= 0
    out[pos] = (np.float32(1.0) / (np.float32(1.0) + np.exp(-x[pos]))).astype(np.float32)
    ex = np.exp(x[~pos]).astype(np.float32)
    out[~pos] = (ex / (np.float32(1.0) + ex)).astype(np.float32)
    return out


def _host_post(results, bh, t_steps=T):
    """Assemble full outputs from per-core results."""
    f32 = np.float32
    decs = np.zeros((BATCH, t_steps), np.int32)
    lps = np.zeros((BATCH, t_steps), f32)
    for c in range(len(results)):
        r = results[c]
        dec_c = np.asarray(r["dec_out"]).T.astype(f32)  # [256, T]
        lgt = np.asarray(r["lgt_out"])  # [T, 256]
        logit = (lgt.T.astype(f32) + f32(bh)).astype(f32)  # [256, T]
        prob = _sigmoid_f32(logit)
        lp1 = np.log(prob + f32(EPS)).astype(f32)
        lp0 = np.log((f32(1.0) - prob) + f32(EPS)).astype(f32)
        lp_c = np.where(dec_c == 1.0, lp1, lp0).astype(f32)
        decs[c * BCORE : (c + 1) * BCORE, :] = dec_c.astype(np.int32)
        lps[c * BCORE : (c + 1) * BCORE, :] = lp_c
    return decs, lps


def kernel(embed, w_ih, w_hh, b_ih, b_hh, w_head, b_head, u, **run_kwargs):
    from concourse.bass_utils import run_bass_kernel_spmd

    nc = _get_program(T)
    in_maps, bh = _host_prep(embed, w_ih, w_hh, b_ih, b_hh, w_head, b_head, u, T)
    res = run_bass_kernel_spmd(nc, in_maps, core_ids=list(range(NCORES)), **run_kwargs)
    out = _host_post(res.results, bh, T)
    kernel.last_results = res
    return out

